# revision 10
# baseline (speedup 1.0000x reference)
"""Trainium2 Bass kernel for CustomBCELoss.

Reference semantics (per torch BCELoss with per-channel weighting):
    p, t flattened channel-first to (C=3, M=8388608)
    ones[c]   = count_nonzero(t[c])
    weight[c] = M / max(ones[c], 1)  if ones[c] > 0 else 1000.0
    bce[c]    = -mean(t*max(log p, -100) + (1-t)*max(log1p(-p), -100))
    out       = mean(weight * bce)

Since t ∈ {0,1}, the per-element term is log|p + t - 1|, and with
p ∈ [1e-4, 1-1e-4] (post-sigmoid probabilities) the -100 clamp never
fires: |p + t - 1| >= ~6e-5 so log >= ~-10.

8-way data-parallel over the flat element range. Per-core engine split,
sized so the ~430 GB/s/core HBM stream (25.2 MB) stays the bottleneck:
    GpSimd DGE : p-tile loads        Sync DGE : t-tile loads
    PE   : per-segment positive counts (ones[128,1].T @ t into PSUM)
    DVE  : d = (p - 1) + t (fused), |d| for even tiles (abs_max, 2x mode)
    ACT  : |d| for odd tiles, Ln(|d|) with fused per-partition accum_out
Tiles never cross an (n, c) half-block boundary, so per-tile/per-segment
partials map 1:1 to channels on the host, which applies the tiny
weight/mean epilogue in float64. Final tiles are tapered to shorten the
pipeline-drain tail.
"""

import numpy as np

import concourse.bacc as bacc
import concourse.bass as bass
import concourse.tile as tile
from concourse import mybir
from concourse.bass_utils import run_bass_kernel_spmd

N_CORES = 8
C = 3
SPATIAL = 128 * 128 * 128            # elements per (n, c) block
N_BATCH = 4
FULL = N_BATCH * C * SPATIAL         # 25_165_824 total elements
PER_CORE = FULL // N_CORES           # 3_145_728
P = 128
# Per-partition column counts per tile; sum must equal PER_CORE / P = 24576.
TILE_F = [4096, 4096, 4096, 4096, 4096, 2048, 2048]
NTILES = len(TILE_F)
TILE_ELEMS = [P * f for f in TILE_F]
assert sum(TILE_ELEMS) == PER_CORE
HALF_BLOCK_COLS = (SPATIAL // 2) // P          # 8192 cols per half-block
N_SEG = (PER_CORE // P) // HALF_BLOCK_COLS     # 3 segments per core
MM_N = 512                                      # matmul moving free dim
M_PER_CH = FULL // C                 # 8_388_608
EMPTY_WEIGHT = 1000.0

_NC_CACHE = None


def _build_nc():
    nc = bacc.Bacc(
        "TRN2", target_bir_lowering=False, debug=False, num_devices=N_CORES
    )
    p_in = nc.declare_dram_parameter(
        "p_in", [PER_CORE], mybir.dt.float32, isOutput=False
    )
    t_in = nc.declare_dram_parameter(
        "t_in", [PER_CORE], mybir.dt.float32, isOutput=False
    )
    vsum_out = nc.declare_dram_parameter(
        "vsum", [P, NTILES], mybir.dt.float32, isOutput=True
    )
    tsum_out = nc.declare_dram_parameter(
        "tsum", [1, N_SEG * MM_N], mybir.dt.float32, isOutput=True
    )

    # segment of each tile + matmul start/stop bookkeeping
    seg_of_tile = []
    off = 0
    for f in TILE_F:
        assert off // HALF_BLOCK_COLS == (off + f - 1) // HALF_BLOCK_COLS
        seg_of_tile.append(off // HALF_BLOCK_COLS)
        off += f
    mm_total = {s: 0 for s in range(N_SEG)}
    for i, f in enumerate(TILE_F):
        mm_total[seg_of_tile[i]] += f // MM_N

    with tile.TileContext(nc) as tc:
        with (
            tc.tile_pool(name="io", bufs=3) as io_pool,
            tc.tile_pool(name="act", bufs=2) as act_pool,
            tc.tile_pool(name="res", bufs=1) as res_pool,
            tc.tile_pool(name="ps", bufs=1, space="PSUM") as ps_pool,
        ):
            ones_t = res_pool.tile([P, 1], mybir.dt.float32)
            nc.vector.memset(ones_t, 1.0)
            vsum_t = res_pool.tile([P, NTILES], mybir.dt.float32)
            cnt_sb = res_pool.tile([1, N_SEG * MM_N], mybir.dt.float32)
            psum_seg = [
                ps_pool.tile(
                    [1, MM_N], mybir.dt.float32, tag=f"seg{s}", name=f"psum_seg{s}"
                )
                for s in range(N_SEG)
            ]
            mm_done = {s: 0 for s in range(N_SEG)}
            off = 0
            for i, f in enumerate(TILE_F):
                n = P * f
                p_src = p_in[off : off + n].rearrange("(p f) -> p f", p=P)
                t_src = t_in[off : off + n].rearrange("(p f) -> p f", p=P)
                off += n
                s = seg_of_tile[i]
                p_t = io_pool.tile([P, f], mybir.dt.float32, tag="p")
                t_t = io_pool.tile([P, f], mybir.dt.float32, tag="t")
                u_t = act_pool.tile([P, f], mybir.dt.float32, tag="u")
                nc.gpsimd.dma_start(out=p_t, in_=p_src)
                nc.sync.dma_start(out=t_t, in_=t_src)
                # per-segment positive counts on the otherwise idle PE
                for j in range(f // MM_N):
                    nc.tensor.matmul(
                        out=psum_seg[s][:, :],
                        lhsT=ones_t[:, :],
                        rhs=t_t[:, j * MM_N : (j + 1) * MM_N],
                        start=(mm_done[s] == 0),
                        stop=(mm_done[s] == mm_total[s] - 1),
                    )
                    mm_done[s] += 1
                # d = (p - 1) + t, in place into p_t
                nc.vector.scalar_tensor_tensor(
                    out=p_t,
                    in0=p_t,
                    scalar=1.0,
                    in1=t_t,
                    op0=mybir.AluOpType.subtract,
                    op1=mybir.AluOpType.add,
                )
                nc.scalar.activation(
                    out=u_t, in_=p_t, func=mybir.ActivationFunctionType.Abs
                )
                ln_in = u_t
                nc.scalar.activation(
                    out=u_t,
                    in_=ln_in,
                    func=mybir.ActivationFunctionType.Ln,
                    accum_out=vsum_t[:, i : i + 1],
                )
            for s in range(N_SEG):
                nc.vector.tensor_copy(
                    out=cnt_sb[:, s * MM_N : (s + 1) * MM_N], in_=psum_seg[s]
                )
            nc.sync.dma_start(out=vsum_out[:], in_=vsum_t)
            nc.sync.dma_start(out=tsum_out[:], in_=cnt_sb)
    nc.compile()
    return nc


def _get_nc():
    global _NC_CACHE
    if _NC_CACHE is None:
        _NC_CACHE = _build_nc()
    return _NC_CACHE


def _run_device(input, target, **spmd_kwargs):
    p_flat = np.ascontiguousarray(input, dtype=np.float32).reshape(-1)
    t_flat = np.ascontiguousarray(target, dtype=np.float32).reshape(-1)
    in_maps = []
    for k in range(N_CORES):
        sl = slice(k * PER_CORE, (k + 1) * PER_CORE)
        in_maps.append({"p_in": p_flat[sl], "t_in": t_flat[sl]})
    return run_bass_kernel_spmd(nc=_get_nc(), in_maps=in_maps,
                                core_ids=list(range(N_CORES)), **spmd_kwargs)


def _epilogue(results):
    sum_v = np.zeros(C, dtype=np.float64)
    sum_t = np.zeros(C, dtype=np.float64)
    for k in range(N_CORES):
        vs = results[k]["vsum"].astype(np.float64)   # [P, NTILES]
        ts = results[k]["tsum"].astype(np.float64)   # [1, N_SEG*MM_N]
        off = 0
        for i, n in enumerate(TILE_ELEMS):
            g = k * PER_CORE + off
            off += n
            ch = (g // SPATIAL) % C
            sum_v[ch] += vs[:, i].sum()
        for s in range(N_SEG):
            ch = ((k * N_SEG + s) // 2) % C
            sum_t[ch] += ts[0, s * MM_N : (s + 1) * MM_N].sum()
    total = float(M_PER_CH)
    ones = sum_t
    weight = np.where(ones > 0, total / np.maximum(ones, 1.0), EMPTY_WEIGHT)
    bce = -sum_v / total
    return np.asarray((weight * bce).mean(), dtype=np.float32)


def kernel(input, target):
    res = _run_device(input, target)
    return _epilogue(res.results)


# revision 12
# speedup vs baseline: 1.0597x; 1.0597x over previous
"""Trainium2 Bass kernel for CustomBCELoss.

Reference semantics (per torch BCELoss with per-channel weighting):
    p, t flattened channel-first to (C=3, M=8388608)
    ones[c]   = count_nonzero(t[c])
    weight[c] = M / max(ones[c], 1)  if ones[c] > 0 else 1000.0
    bce[c]    = -mean(t*max(log p, -100) + (1-t)*max(log1p(-p), -100))
    out       = mean(weight * bce)

Since t ∈ {0,1}, the per-element term is log|p + t - 1|, and with
p ∈ [1e-4, 1-1e-4] (post-sigmoid probabilities) the -100 clamp never
fires: |p + t - 1| >= ~6e-5 so log >= ~-10.

8-way data-parallel over the flat element range. Per-core engine split,
sized so the ~430 GB/s/core HBM stream (25.2 MB) stays the bottleneck:
    GpSimd DGE : p-tile loads        Sync DGE : t-tile loads
    PE   : per-segment positive counts (ones[128,1].T @ t into PSUM)
    DVE  : d = (p - 1) + t (fused), |d| for even tiles (abs_max, 2x mode)
    ACT  : |d| for odd tiles, Ln(|d|) with fused per-partition accum_out
Tiles never cross an (n, c) half-block boundary, so per-tile/per-segment
partials map 1:1 to channels on the host, which applies the tiny
weight/mean epilogue in float64. Final tiles are tapered to shorten the
pipeline-drain tail.
"""

import numpy as np

import concourse.bacc as bacc
import concourse.bass as bass
import concourse.tile as tile
from concourse import mybir
from concourse.bass_utils import run_bass_kernel_spmd

N_CORES = 8
C = 3
SPATIAL = 128 * 128 * 128            # elements per (n, c) block
N_BATCH = 4
FULL = N_BATCH * C * SPATIAL         # 25_165_824 total elements
PER_CORE = FULL // N_CORES           # 3_145_728
P = 128
# Per-partition column counts per tile; sum must equal PER_CORE / P = 24576.
TILE_F = [4096, 4096, 4096, 4096, 4096, 2048, 2048]
NTILES = len(TILE_F)
TILE_ELEMS = [P * f for f in TILE_F]
assert sum(TILE_ELEMS) == PER_CORE
HALF_BLOCK_COLS = (SPATIAL // 2) // P          # 8192 cols per half-block
N_SEG = (PER_CORE // P) // HALF_BLOCK_COLS     # 3 segments per core
MM_N = 512                                      # matmul moving free dim
M_PER_CH = FULL // C                 # 8_388_608
EMPTY_WEIGHT = 1000.0

_NC_CACHE = None


def _build_nc():
    nc = bacc.Bacc(
        "TRN2", target_bir_lowering=False, debug=False, num_devices=N_CORES
    )
    p_in = nc.declare_dram_parameter(
        "p_in", [PER_CORE], mybir.dt.float32, isOutput=False
    )
    t_in = nc.declare_dram_parameter(
        "t_in", [PER_CORE], mybir.dt.float32, isOutput=False
    )
    vsum_out = nc.declare_dram_parameter(
        "vsum", [P, NTILES], mybir.dt.float32, isOutput=True
    )
    tsum_out = nc.declare_dram_parameter(
        "tsum", [1, N_SEG * MM_N], mybir.dt.float32, isOutput=True
    )

    # segment of each tile + matmul start/stop bookkeeping
    seg_of_tile = []
    off = 0
    for f in TILE_F:
        assert off // HALF_BLOCK_COLS == (off + f - 1) // HALF_BLOCK_COLS
        seg_of_tile.append(off // HALF_BLOCK_COLS)
        off += f
    mm_total = {s: 0 for s in range(N_SEG)}
    for i, f in enumerate(TILE_F):
        mm_total[seg_of_tile[i]] += f // MM_N

    with tile.TileContext(nc) as tc:
        with (
            tc.tile_pool(name="io", bufs=3) as io_pool,
            tc.tile_pool(name="act", bufs=2) as act_pool,
            tc.tile_pool(name="res", bufs=1) as res_pool,
            tc.tile_pool(name="ps", bufs=1, space="PSUM") as ps_pool,
        ):
            ones_t = res_pool.tile([P, 1], mybir.dt.bfloat16)
            nc.vector.memset(ones_t, 1.0)
            vsum_t = res_pool.tile([P, NTILES], mybir.dt.float32)
            cnt_sb = res_pool.tile([1, N_SEG * MM_N], mybir.dt.float32)
            psum_seg = [
                ps_pool.tile(
                    [1, MM_N], mybir.dt.float32, tag=f"seg{s}", name=f"psum_seg{s}"
                )
                for s in range(N_SEG)
            ]
            mm_done = {s: 0 for s in range(N_SEG)}
            off = 0
            for i, f in enumerate(TILE_F):
                n = P * f
                p_src = p_in[off : off + n].rearrange("(p f) -> p f", p=P)
                t_src = t_in[off : off + n].rearrange("(p f) -> p f", p=P)
                off += n
                s = seg_of_tile[i]
                p_t = io_pool.tile([P, f], mybir.dt.float32, tag="p")
                t_t = io_pool.tile([P, f], mybir.dt.float32, tag="t")
                u_t = act_pool.tile([P, f], mybir.dt.float32, tag="u")
                nc.gpsimd.dma_start(out=p_t, in_=p_src)
                nc.sync.dma_start(out=t_t, in_=t_src)
                # per-segment positive counts on the otherwise idle PE.
                # t is exactly 0.0f/1.0f; its high 2 bytes viewed as bf16 are
                # exactly 0.0/1.0, so a strided bf16 view avoids the 2-pass
                # fp32 matmul while keeping the count exact.
                t_hi = t_t[:].bitcast(mybir.dt.bfloat16).rearrange(
                    "p (f two) -> p f two", two=2
                )[:, :, 1]
                for j in range(f // MM_N):
                    nc.tensor.matmul(
                        out=psum_seg[s][:, :],
                        lhsT=ones_t[:, :],
                        rhs=t_hi[:, j * MM_N : (j + 1) * MM_N],
                        start=(mm_done[s] == 0),
                        stop=(mm_done[s] == mm_total[s] - 1),
                    )
                    mm_done[s] += 1
                # d = (p - 1) + t, in place into p_t
                nc.vector.scalar_tensor_tensor(
                    out=p_t,
                    in0=p_t,
                    scalar=1.0,
                    in1=t_t,
                    op0=mybir.AluOpType.subtract,
                    op1=mybir.AluOpType.add,
                )
                nc.scalar.activation(
                    out=u_t, in_=p_t, func=mybir.ActivationFunctionType.Abs
                )
                ln_in = u_t
                nc.scalar.activation(
                    out=u_t,
                    in_=ln_in,
                    func=mybir.ActivationFunctionType.Ln,
                    accum_out=vsum_t[:, i : i + 1],
                )
            for s in range(N_SEG):
                nc.vector.tensor_copy(
                    out=cnt_sb[:, s * MM_N : (s + 1) * MM_N], in_=psum_seg[s]
                )
            nc.sync.dma_start(out=vsum_out[:], in_=vsum_t)
            nc.sync.dma_start(out=tsum_out[:], in_=cnt_sb)
    nc.compile()
    return nc


def _get_nc():
    global _NC_CACHE
    if _NC_CACHE is None:
        _NC_CACHE = _build_nc()
    return _NC_CACHE


def _run_device(input, target, **spmd_kwargs):
    p_flat = np.ascontiguousarray(input, dtype=np.float32).reshape(-1)
    t_flat = np.ascontiguousarray(target, dtype=np.float32).reshape(-1)
    in_maps = []
    for k in range(N_CORES):
        sl = slice(k * PER_CORE, (k + 1) * PER_CORE)
        in_maps.append({"p_in": p_flat[sl], "t_in": t_flat[sl]})
    return run_bass_kernel_spmd(nc=_get_nc(), in_maps=in_maps,
                                core_ids=list(range(N_CORES)), **spmd_kwargs)


def _epilogue(results):
    sum_v = np.zeros(C, dtype=np.float64)
    sum_t = np.zeros(C, dtype=np.float64)
    for k in range(N_CORES):
        vs = results[k]["vsum"].astype(np.float64)   # [P, NTILES]
        ts = results[k]["tsum"].astype(np.float64)   # [1, N_SEG*MM_N]
        off = 0
        for i, n in enumerate(TILE_ELEMS):
            g = k * PER_CORE + off
            off += n
            ch = (g // SPATIAL) % C
            sum_v[ch] += vs[:, i].sum()
        for s in range(N_SEG):
            ch = ((k * N_SEG + s) // 2) % C
            sum_t[ch] += ts[0, s * MM_N : (s + 1) * MM_N].sum()
    total = float(M_PER_CH)
    ones = sum_t
    weight = np.where(ones > 0, total / np.maximum(ones, 1.0), EMPTY_WEIGHT)
    bce = -sum_v / total
    return np.asarray((weight * bce).mean(), dtype=np.float32)


def kernel(input, target):
    res = _run_device(input, target)
    return _epilogue(res.results)
